# revision 11
# baseline (speedup 1.0000x reference)
"""Trainium2 Bass kernel for a 2-layer Mistral-style VLM block (tensor-parallel, 8 cores).

Strategy (v2):
- LoRA rank-8 folded into base weights on host (float64, exact up to f32 rounding).
- ln1/ln2 folded into Wq/Wk/Wv and Wg/Wu columns on host.
- Tensor parallel: Wq/Wk/Wv/Wg/Wu sharded on output dim, Wo/Wd on input dim.
  Core r holds Q heads 4r..4r+3 + KV head r -> attention fully local.
- Activations feature-major on chip (d on partitions, tokens on free dim).
- fp16 weights (half the HBM traffic of f32r, ~1 extra mantissa bit lost),
  fp16 activations (xmega/amega/mts), f32 accumulation everywhere; softmax
  exp table kept f32r for range safety; residual stream h kept f32 in DRAM.
- rmsnorm folded: per-token scale applied to q/k (fused into rope epilogue), v, g, u.
- Per-batch (768-token) chunking; fp16 AllReduce (Shared outputs) after Wo/Wd
  partials per batch, overlapped with the other batch's compute.
- Projector output-sharded + fp16 AllGather (instead of K-shard + AllReduce).
- MLP single-pass: all 14 gate/up tiles resident in SBUF (fp16), one down-proj
  contraction, no DMA-accumulate.
- Weight k-slabs fetched as single large DMAs ([128, 16*128] per descriptor).
- Output written fp16, upcast on host.
"""

import sys

sys.path.insert(0, '/opt/trn_rl_repo')

import numpy as np
import ml_dtypes

NCORES = 8
D, VH, DFF, NL, VOCAB, NH, NKV, HD, RK, SCALE = 4096, 1024, 14336, 2, 32000, 32, 8, 128, 8, 4.0
B, NIMG, T = 2, 257, 511
S = NIMG + T            # 768
NTOK = B * S            # 1536
DSH = D // NCORES       # 512
FSH = DFF // NCORES     # 1792
KT = D // 128           # 32
FT = FSH // 128         # 14
QH = NH // NCORES       # 4
CH = 384
NCH = S // CH           # 2
JT = S // 128           # 6
VKT = VH // 128         # 8
PM = DSH // 128         # 4 projector out tiles per core
EPS = 1e-5
ISQ = 1.0 / float(np.sqrt(HD))
EXP_BIAS = -10.0
MASK_NEG = -1e30
NIMGP = NIMG + 1          # pad to even free size for fp32r matmul

BF16 = ml_dtypes.bfloat16
_PROGRAM = None


def _bf(x):
    return np.ascontiguousarray(np.asarray(x, np.float32).astype(BF16))


def _f16(x):
    return np.ascontiguousarray(np.asarray(x, np.float32).astype(np.float16))


def _r(x):
    """fp32 -> fp32r RNE rounding (11 explicit mantissa bits); bit-exact vs HW cast."""
    u = np.ascontiguousarray(x, np.float32).view(np.uint32)
    low = u & np.uint32(0xFFF)
    hi = u >> np.uint32(12)
    carry = (low > 0x800) | ((low == 0x800) & ((hi & 1) == 1))
    return ((hi + carry.astype(np.uint32)) << np.uint32(12)).view(np.float32)


def _build_program():
    import concourse.bass as bass
    import concourse.bacc as bacc
    import concourse.mybir as mybir
    import concourse.tile as tile

    F32 = mybir.dt.float32
    F32R = mybir.dt.float32r
    F16 = mybir.dt.float16
    BF = mybir.dt.bfloat16
    AF = mybir.ActivationFunctionType
    ALU = mybir.AluOpType

    nc = bacc.Bacc("TRN2", target_bir_lowering=False)

    img_in = nc.dram_tensor("img", [128, VKT * B * NIMGP], F32R, kind="ExternalInput")
    projw_in = nc.dram_tensor("projw", [PM, 128, VKT * 128], F16, kind="ExternalInput")
    projb_in = nc.dram_tensor("projb", [128, KT], F32, kind="ExternalInput")
    txt_in = nc.dram_tensor("txt", [D, B * T], F32, kind="ExternalInput")
    cos_in = nc.dram_tensor("cos_t", [128, S], F32, kind="ExternalInput")
    sin_in = nc.dram_tensor("sin_t", [128, S], F32, kind="ExternalInput")   # sign-folded
    mask_in = nc.dram_tensor("mask6", [6, 128, CH], BF, kind="ExternalInput")
    onesr_in = nc.dram_tensor("onesr", [128, 1], F32R, kind="ExternalInput")
    onesb_in = nc.dram_tensor("onesb", [128, 1], BF, kind="ExternalInput")
    ident_in = nc.dram_tensor("ident", [128, 128], F16, kind="ExternalInput")
    lnf_in = nc.dram_tensor("lnf", [128, KT], F32, kind="ExternalInput")
    wqkv_in = [nc.dram_tensor(f"wqkv{l}", [6, 128, KT * 128], F16, kind="ExternalInput") for l in range(NL)]
    wo_in = [nc.dram_tensor(f"wo{l}", [KT, 128, QH * 128], F16, kind="ExternalInput") for l in range(NL)]
    wgu_in = [nc.dram_tensor(f"wgu{l}", [FT, 2, 128, KT * 128], F16, kind="ExternalInput") for l in range(NL)]
    wd_in = [nc.dram_tensor(f"wd{l}", [KT, 128, FT * 128], F16, kind="ExternalInput") for l in range(NL)]
    out_ext = nc.dram_tensor("out", [D, NTOK], F16, kind="ExternalOutput")

    RG = [list(range(NCORES))]

    with tile.TileContext(nc) as tc:
        with tc.tile_pool(name="sb", bufs=1) as sb, \
             tc.tile_pool(name="ps", bufs=1, space="PSUM") as ps, \
             tc.tile_pool(name="dram", bufs=1, space="DRAM") as dram:

            # ---- resident constants ----
            cos_sb = sb.tile([128, S], F32, tag="res_cos", bufs=1)
            sin_sb = sb.tile([128, S], F32, tag="res_sin", bufs=1)
            onesr_sb = sb.tile([128, 1], F32R, tag="res_onesr", bufs=1)
            onesb_sb = sb.tile([128, 1], BF, tag="res_onesb", bufs=1)
            ident_sb = sb.tile([128, 128], F16, tag="res_ident", bufs=1)
            projb_sb = sb.tile([128, KT], F32, tag="res_projb", bufs=1)
            lnf_sb = sb.tile([128, KT], F32, tag="res_lnf", bufs=1)
            for t_, i_ in [(cos_sb, cos_in), (sin_sb, sin_in), (onesr_sb, onesr_in),
                           (onesb_sb, onesb_in), (ident_sb, ident_in),
                           (projb_sb, projb_in), (lnf_sb, lnf_in)]:
                nc.sync.dma_start(t_[:], i_[:])
            mask_sb = []
            for j in range(6):
                mt = sb.tile([128, CH], BF, tag=f"res_mask{j}", bufs=1)
                nc.sync.dma_start(mt[:], mask_in[j])
                mask_sb.append(mt)
            eps_sb = sb.tile([128, 1], F32, tag="res_eps", bufs=1)
            nb_sb = sb.tile([128, 1], F32, tag="res_nb", bufs=1)
            nc.vector.memset(eps_sb[:], EPS)
            nc.vector.memset(nb_sb[:], EXP_BIAS)

            h_d = [dram.tile([D, S], F32, tag=f"hdram{b}", bufs=1, name=f"h_d{b}") for b in range(B)]

            # ---- phase 0: txt DMA + output-sharded projector + AllGather ----
            nc.sync.dma_start(h_d[0][:, NIMG:S], txt_in[:, 0:T])
            nc.sync.dma_start(h_d[1][:, NIMG:S], txt_in[:, T:2 * T])

            # full image activations resident (shares the xmega slot; phase 0 only)
            img_sb = sb.tile([128, VKT * B * NIMGP], F32R, tag="xmega", bufs=1, name="img_sb")
            nc.sync.dma_start(img_sb[:], img_in[:])
            arin_p = dram.tile([DSH, B * NIMG], F16, tag="arinp", bufs=1, name="arin_p")
            ag_out = dram.tile([D, B * NIMG], F16, tag="agout", bufs=1,
                               addr_space="Shared", name="ag_out")
            for m in range(PM):
                pw = sb.tile([128, VKT * 128], F16, tag="wbig", bufs=3, name=f"pw{m}")
                nc.sync.dma_start(pw[:], projw_in[m])
                for b in range(B):
                    pt = ps.tile([128, NIMGP], F32, tag="ps1", bufs=4, name=f"pj{m}{b}")
                    for k in range(VKT):
                        nc.tensor.matmul(
                            pt[:], pw[:, k * 128:(k + 1) * 128],
                            img_sb[:, (k * B + b) * NIMGP:(k * B + b + 1) * NIMGP],
                            start=(k == 0), stop=(k == VKT - 1))
                    ev = sb.tile([128, CH], F16, tag="evac", bufs=2, name=f"pje{m}{b}")
                    nc.scalar.activation(ev[:, :NIMG], pt[:, :NIMG], AF.Copy)
                    nc.sync.dma_start(arin_p[m * 128:(m + 1) * 128, b * NIMG:(b + 1) * NIMG],
                                      ev[:, :NIMG])
            nc.gpsimd.collective_compute("AllGather", ALU.bypass, replica_groups=RG,
                                         ins=[arin_p.opt()], outs=[ag_out.opt()])
            for m in range(KT):
                hi = sb.tile([128, S], F16, tag="arttmp", bufs=3, name=f"pha{m}")
                nc.sync.dma_start(hi[:, :B * NIMG], ag_out[m * 128:(m + 1) * 128, :])
                hb = sb.tile([128, S], F32, tag="tmp", bufs=4, name=f"phb{m}")
                nc.scalar.activation(hb[:, :B * NIMG], hi[:, :B * NIMG], AF.Identity,
                                     bias=projb_sb[:, m:m + 1])
                for b in range(B):
                    nc.sync.dma_start(h_d[b][m * 128:(m + 1) * 128, 0:NIMG],
                                      hb[:, b * NIMG:(b + 1) * NIMG])

            # =========================================================
            def bcast_row(row_ap, width, tag, nm):
                rd = dram.tile([1, width], F32, tag="rowd", bufs=4, name=f"rd{nm}")
                nc.sync.dma_start(rd[:], row_ap)
                bc = sb.tile([128, width], F32, tag=tag, bufs=1, name=f"bct{nm}")
                nc.sync.dma_start(bc[:], rd[:].to_broadcast((128, width)))
                return bc

            def load_slab(w_l, m, klo, khi, nm):
                """Stream lhsT k-tiles [klo,khi) for output tile m as <=16-tile slabs,
                one contiguous DMA per slab (weights stored [m, p, k*128+f])."""
                slabs = []
                lo = klo
                while lo < khi:
                    hi = min(khi, lo + 16)
                    sl = sb.tile([128, 2048], F16, tag="wbig", bufs=3, name=f"sl{nm}{m}_{lo}")
                    nc.sync.dma_start(sl[:, :(hi - lo) * 128],
                                      w_l[m][:, lo * 128:hi * 128])
                    slabs.append((sl, lo, hi))
                    lo = hi
                return slabs

            def slab_ap(slabs, k):
                for sl, lo, hi in slabs:
                    if lo <= k < hi:
                        return sl[:, (k - lo) * 128:(k - lo + 1) * 128]
                raise KeyError(k)

            def norm_prep(b, l, site, ar_tile, writeback=True):
                """x (f16) + 1/rms broadcast. Updates h_d[b] if ar_tile given."""
                xmega = sb.tile([128, KT * S], F16, tag="xmega", bufs=1, name=f"x{l}{site}{b}")
                ssq_ps = [ps.tile([1, CH], F32, tag="psS", bufs=2, name=f"sq{l}{site}{b}{c}")
                          for c in range(NCH)]
                for k in range(KT):
                    rows = slice(k * 128, (k + 1) * 128)
                    xk = xmega[:, k * S:(k + 1) * S]
                    if ar_tile is not None:
                        hold = sb.tile([128, S], F32, tag="tmp", bufs=4, name=f"ho{l}{site}{b}{k}")
                        art = sb.tile([128, S], F16, tag="arttmp", bufs=3, name=f"ar{l}{site}{b}{k}")
                        nc.sync.dma_start(hold[:], h_d[b][rows, :])
                        nc.sync.dma_start(art[:], ar_tile[rows, :])
                        hnew = sb.tile([128, S], F32, tag="tmp", bufs=4, name=f"hn{l}{site}{b}{k}")
                        nc.vector.tensor_tensor(hnew[:], hold[:], art[:], ALU.add)
                        if writeback:
                            nc.sync.dma_start(h_d[b][rows, :], hnew[:])
                        nc.scalar.activation(xk, hnew[:], AF.Copy)      # f32 -> f16 cast
                        sqsrc = hnew
                    else:
                        hnew = sb.tile([128, S], F32, tag="tmp", bufs=4, name=f"hz{l}{site}{b}{k}")
                        nc.sync.dma_start(hnew[:], h_d[b][rows, :])
                        nc.scalar.activation(xk, hnew[:], AF.Copy)
                        sqsrc = hnew
                    sq = sb.tile([128, S], BF, tag="sq", bufs=2, name=f"s{l}{site}{b}{k}")
                    nc.scalar.activation(sq[:], sqsrc[:], AF.Square)
                    for c in range(NCH):
                        nc.tensor.matmul(ssq_ps[c][:], onesb_sb[:],
                                         sq[:, c * CH:(c + 1) * CH],
                                         start=(k == 0), stop=(k == KT - 1))
                s_sb = sb.tile([1, S], F32, tag="scal", bufs=2, name=f"ss{l}{site}{b}")
                r_sb = sb.tile([1, S], F32, tag="scal", bufs=2, name=f"sr{l}{site}{b}")
                for c in range(NCH):
                    nc.scalar.activation(s_sb[:, c * CH:(c + 1) * CH], ssq_ps[c][:],
                                         AF.Sqrt, scale=1.0 / D, bias=eps_sb[0:1, :])
                nc.vector.reciprocal(r_sb[:], s_sb[:])
                bc = bcast_row(r_sb[:], S, "bc", f"n{l}{site}{b}")
                return xmega, bc

            def qkv_attn(b, l, xmega, bc):
                """QKV + rope + attention -> amega (128, QH*S) f16 resident."""
                qkmega = sb.tile([128, 5 * S], F16, tag="qkmega", bufs=2, name=f"qkm{l}{b}")
                vsb = sb.tile([128, S], F16, tag="vsb", bufs=2, name=f"v{l}{b}")
                for m in range(6):
                    slabs = load_slab(wqkv_in[l], m, 0, KT, f"q{l}{b}")
                    if m < 5:
                        qraw = sb.tile([128, S], F32, tag="tmp", bufs=4, name=f"qr{l}{b}{m}")
                    for c in range(NCH):
                        cs_ = slice(c * CH, (c + 1) * CH)
                        pt = ps.tile([128, CH], F32, tag="ps1", bufs=4, name=f"qp{l}{b}{m}{c}")
                        for k in range(KT):
                            nc.tensor.matmul(
                                pt[:], slab_ap(slabs, k),
                                xmega[:, k * S + c * CH: k * S + (c + 1) * CH],
                                start=(k == 0), stop=(k == KT - 1))
                        if m < 5:
                            nc.scalar.activation(qraw[:, cs_], pt[:], AF.Copy)
                        else:
                            nc.vector.tensor_tensor(vsb[:, cs_], pt[:], bc[:, cs_], ALU.mult)
                    if m < 5:
                        qs = sb.tile([128, S], F32, tag="tmp", bufs=4, name=f"qh{l}{b}{m}")
                        nc.sync.dma_start(qs[0:64, :], qraw[64:128, :])
                        nc.sync.dma_start(qs[64:128, :], qraw[0:64, :])
                        t2 = sb.tile([128, S], F32, tag="tmp", bufs=4, name=f"t2{l}{b}{m}")
                        nc.vector.tensor_tensor(t2[:], qraw[:], cos_sb[:], ALU.mult)
                        u2 = sb.tile([128, S], F32, tag="tmp", bufs=4, name=f"u2{l}{b}{m}")
                        nc.vector.tensor_tensor(u2[:], qs[:], sin_sb[:], ALU.mult)
                        q3 = sb.tile([128, S], F32, tag="tmp", bufs=4, name=f"q3{l}{b}{m}")
                        nc.vector.tensor_tensor(q3[:], t2[:], u2[:], ALU.add)
                        nc.vector.tensor_tensor(qkmega[:, m * S:(m + 1) * S],
                                                q3[:], bc[:], ALU.mult)

                vtok = []
                for t in range(JT):
                    trp = ps.tile([128, 128], F16, tag="ps1", bufs=4, name=f"vt{l}{b}{t}")
                    nc.tensor.transpose(trp[:], vsb[:, t * 128:(t + 1) * 128], ident_sb[:])
                    vt = sb.tile([128, 128], F16, tag="vtok", bufs=6, name=f"vk{l}{b}{t}")
                    nc.scalar.activation(vt[:], trp[:], AF.Copy)
                    vtok.append(vt)

                amega = sb.tile([128, QH * S], F16, tag="amega", bufs=1, name=f"am{l}{b}")
                ksb = qkmega[:, 4 * S:5 * S]
                for hh in range(QH):
                    qh_t = qkmega[:, hh * S:(hh + 1) * S]
                    for c in range(NCH):
                        njt = 3 * (c + 1)
                        ap_ps = ps.tile([128, CH], F32, tag="psA", bufs=2, name=f"ap{l}{b}{hh}{c}")
                        ss_ps = ps.tile([1, CH], F32, tag="psS", bufs=2, name=f"sm{l}{b}{hh}{c}")
                        for jt in range(njt):
                            sc = ps.tile([128, CH], F32, tag="ps1", bufs=4, name=f"sc{l}{b}{hh}{c}{jt}")
                            nc.tensor.matmul(sc[:], ksb[:, jt * 128:(jt + 1) * 128],
                                             qh_t[:, c * CH:(c + 1) * CH],
                                             start=True, stop=True)
                            et = sb.tile([128, CH], F32R, tag="expT", bufs=2, name=f"et{l}{b}{hh}{c}{jt}")
                            if jt >= 3 * c:
                                madd = sb.tile([128, CH], F32, tag="madd", bufs=1, name=f"md{l}{b}{hh}{c}{jt}")
                                nc.vector.tensor_tensor(madd[:], sc[:], mask_sb[jt][:], ALU.add)
                                nc.scalar.activation(et[:], madd[:], AF.Exp, scale=ISQ, bias=nb_sb[:])
                            else:
                                nc.scalar.activation(et[:], sc[:], AF.Exp, scale=ISQ, bias=nb_sb[:])
                            nc.tensor.matmul(ss_ps[:], onesr_sb[:], et[:],
                                             start=(jt == 0), stop=(jt == njt - 1))
                            nc.tensor.matmul(ap_ps[:], vtok[jt][:], et[:],
                                             start=(jt == 0), stop=(jt == njt - 1))
                        rec = sb.tile([1, CH], F32, tag="scal", bufs=2, name=f"rc{l}{b}{hh}{c}")
                        nc.vector.reciprocal(rec[:], ss_ps[:])
                        rbc = bcast_row(rec[:], CH, "rbc", f"a{l}{b}{hh}{c}")
                        nc.vector.tensor_tensor(
                            amega[:, hh * S + c * CH: hh * S + (c + 1) * CH],
                            ap_ps[:], rbc[:], ALU.mult)
                return amega

            def shard_gemm_ar(b, l, rhs_tiles, w_l, nkt, site):
                """rhs_tiles(k, c) -> AP for k-tile; fp16 partial (D,S) + AllReduce."""
                arin = dram.tile([D, S], F16, tag="arin", bufs=2, name=f"ai{l}{site}{b}")
                arout = dram.tile([D, S], F16, tag="arout", bufs=2,
                                  addr_space="Shared", name=f"ao{l}{site}{b}")
                for m in range(KT):
                    slabs = load_slab(w_l, m, 0, nkt, f"{site}{l}{b}")
                    for c in range(NCH):
                        pt = ps.tile([128, CH], F32, tag="ps1", bufs=4, name=f"o{l}{site}{b}{m}{c}")
                        for k in range(nkt):
                            nc.tensor.matmul(
                                pt[:], slab_ap(slabs, k),
                                rhs_tiles(k, c),
                                start=(k == 0), stop=(k == nkt - 1))
                        ev = sb.tile([128, CH], F16, tag="evac", bufs=2, name=f"oe{l}{site}{b}{m}{c}")
                        nc.scalar.activation(ev[:], pt[:], AF.Copy)
                        nc.sync.dma_start(arin[m * 128:(m + 1) * 128, c * CH:(c + 1) * CH], ev[:])
                nc.gpsimd.collective_compute("AllReduce", ALU.add, replica_groups=RG,
                                             ins=[arin.opt()], outs=[arout.opt()])
                return arout

            def mlp_wd(b, l, xmega, bc):
                """gate/up gemms + silu*u (fp16, all FT tiles resident) + single
                down-proj contraction. Returns arout."""
                arin = dram.tile([D, S], F16, tag="arin", bufs=2, name=f"aid{l}{b}")
                arout = dram.tile([D, S], F16, tag="arout", bufs=2,
                                  addr_space="Shared", name=f"aod{l}{b}")
                mts = {}
                for j in range(FT):
                    pts = []
                    for gu in range(2):
                        slabs = load_slab(wgu_in[l][j], gu, 0, KT, f"g{l}{b}{j}")
                        for c in range(NCH):
                            pt = ps.tile([128, CH], F32, tag="ps1", bufs=4,
                                         name=f"g{l}{b}{j}{gu}{c}")
                            for k in range(KT):
                                nc.tensor.matmul(
                                    pt[:], slab_ap(slabs, k),
                                    xmega[:, k * S + c * CH: k * S + (c + 1) * CH],
                                    start=(k == 0), stop=(k == KT - 1))
                            pts.append(pt)
                    gsb = sb.tile([128, S], F32, tag="tmp", bufs=4, name=f"gs{l}{b}{j}")
                    usb = sb.tile([128, S], F32, tag="tmp", bufs=4, name=f"us{l}{b}{j}")
                    for c in range(NCH):
                        cs_ = slice(c * CH, (c + 1) * CH)
                        nc.vector.tensor_tensor(gsb[:, cs_], pts[c][:], bc[:, cs_], ALU.mult)
                        nc.vector.tensor_tensor(usb[:, cs_], pts[2 + c][:], bc[:, cs_], ALU.mult)
                    sil = sb.tile([128, S], F32, tag="tmp", bufs=4, name=f"si{l}{b}{j}")
                    nc.scalar.activation(sil[:], gsb[:], AF.Silu)
                    mt = sb.tile([128, S], F16, tag="mstream", bufs=FT + 1,
                                 name=f"mt{l}{b}{j}")
                    nc.vector.tensor_tensor(mt[:], sil[:], usb[:], ALU.mult)
                    mts[j] = mt
                # single down-proj contraction over all FT tiles
                for m in range(KT):
                    slabs = load_slab(wd_in[l], m, 0, FT, f"d{l}{b}")
                    for c in range(NCH):
                        pt = ps.tile([128, CH], F32, tag="ps1", bufs=4,
                                     name=f"dp{l}{b}{m}{c}")
                        for k in range(FT):
                            nc.tensor.matmul(
                                pt[:], slab_ap(slabs, k),
                                mts[k][:, c * CH:(c + 1) * CH],
                                start=(k == 0), stop=(k == FT - 1))
                        ev = sb.tile([128, CH], F16, tag="evac", bufs=2,
                                     name=f"de{l}{b}{m}{c}")
                        nc.scalar.activation(ev[:], pt[:], AF.Copy)
                        nc.sync.dma_start(arin[m * 128:(m + 1) * 128, c * CH:(c + 1) * CH], ev[:])
                nc.gpsimd.collective_compute("AllReduce", ALU.add, replica_groups=RG,
                                             ins=[arin.opt()], outs=[arout.opt()])
                return arout

            def final_norm(b, ar_tile):
                xmega, bc = norm_prep(b, 9, 'f', ar_tile, writeback=False)
                for k in range(KT):
                    rows = slice(k * 128, (k + 1) * 128)
                    ot = sb.tile([128, S], F16, tag="arttmp", bufs=3, name=f"ot{b}{k}")
                    nc.vector.scalar_tensor_tensor(ot[:], xmega[:, k * S:(k + 1) * S],
                                                   lnf_sb[:, k:k + 1], bc[:],
                                                   ALU.mult, ALU.mult)
                    nc.sync.dma_start(out_ext[rows, b * S:(b + 1) * S], ot[:])

            # ---- main schedule ----
            ar_pending = [None, None]
            for l in range(NL):
                ar_o = [None, None]
                for b in range(B):
                    xmega, bc = norm_prep(b, l, 'a', ar_pending[b])
                    amega = qkv_attn(b, l, xmega, bc)
                    ar_o[b] = shard_gemm_ar(
                        b, l, lambda k, c, am=amega: am[:, k * S + c * CH: k * S + (c + 1) * CH],
                        wo_in[l], QH, 'o')
                for b in range(B):
                    xmega, bc = norm_prep(b, l, 'm', ar_o[b])
                    ar_pending[b] = mlp_wd(b, l, xmega, bc)
            for b in range(B):
                final_norm(b, ar_pending[b])

    nc.compile()
    return nc


def _host_prep(inputs):
    I = {k: np.asarray(v) for k, v in inputs.items()}

    def fold(W, A, Bm, lnw=None):
        W64 = W.astype(np.float64) + SCALE * (Bm.astype(np.float64) @ A.astype(np.float64))
        if lnw is not None:
            W64 = W64 * lnw.astype(np.float64)[None, :]
        return W64.astype(np.float32)

    ids = np.asarray(I['input_ids'], np.int64)
    txt = I['embed'][ids]                                    # (B, T, D)
    txtT = np.ascontiguousarray(txt.reshape(B * T, D).T).astype(np.float32)

    inv = 1.0 / (10000.0 ** (np.arange(0, HD, 2, dtype=np.float64) / HD))
    ang = np.arange(S, dtype=np.float64)[:, None] * inv[None, :]
    cosT = np.ascontiguousarray(np.concatenate([np.cos(ang), np.cos(ang)], 1).T).astype(np.float32)
    sinT = np.ascontiguousarray(np.concatenate([-np.sin(ang), np.sin(ang)], 1).T).astype(np.float32)

    mask6 = np.zeros((6, 128, CH), np.float32)
    for jt in range(6):
        c = 0 if jt < 3 else 1
        jj = np.arange(jt * 128, (jt + 1) * 128)[:, None]
        ii = np.arange(c * CH, (c + 1) * CH)[None, :]
        mask6[jt] = np.where(jj <= ii, 0.0, MASK_NEG)

    imgT = np.ascontiguousarray(I['image_embeds'].reshape(B * NIMG, VH).T.astype(np.float32))  # (VH, 514)
    projW = I['proj_W'].astype(np.float32)                   # (D, VH)
    projT = projW.T                                          # (VH, D)
    projb_t = np.ascontiguousarray(I['proj_b'].astype(np.float32).reshape(KT, 128).T)
    lnf_t = np.ascontiguousarray(I['ln_f'].astype(np.float32).reshape(KT, 128).T)

    # full image, padded: [128, VKT*B*NIMGP]; img[p, (k*B+b)*NIMGP + i] = imgT[128k+p, b*NIMG + i]
    imp4 = np.zeros((128, VKT, B, NIMGP), np.float32)
    for b in range(B):
        imp4[:, :, b, :NIMG] = imgT[:, b * NIMG:(b + 1) * NIMG] \
            .reshape(VKT, 128, NIMG).transpose(1, 0, 2)
    imp = imp4.reshape(128, VKT * B * NIMGP)

    shared = dict(
        projb=projb_t, txt=txtT, cos_t=cosT, sin_t=sinT,
        mask6=_bf(mask6),
        img=_r(imp),
        onesr=_r(np.ones((128, 1), np.float32)),
        onesb=_bf(np.ones((128, 1), np.float32)),
        ident=_f16(np.eye(128, dtype=np.float32)),
        lnf=lnf_t,
    )

    per_core = [dict(shared) for _ in range(NCORES)]
    for r in range(NCORES):
        cols = slice(r * DSH, (r + 1) * DSH)
        # projw: [m, p, 128k+f] = projT[128k+p, 512r + 128m + f]
        per_core[r]["projw"] = _f16(np.ascontiguousarray(
            projT[:, cols].reshape(VKT, 128, PM, 128).transpose(2, 1, 0, 3)
            .reshape(PM, 128, VKT * 128)))

    for l in range(NL):
        Wq = fold(I['Wq'][l], I['Aq'][l], I['Bq'][l], I['ln1'][l])
        Wk = fold(I['Wk'][l], I['Ak'][l], I['Bk'][l], I['ln1'][l])
        Wv = fold(I['Wv'][l], I['Av'][l], I['Bv'][l], I['ln1'][l])
        Wo = fold(I['Wo'][l], I['Ao'][l], I['Bo'][l])
        Wg = fold(I['Wg'][l], I['Ag'][l], I['Bg'][l], I['ln2'][l])
        Wu = fold(I['Wu'][l], I['Au'][l], I['Bu'][l], I['ln2'][l])
        Wd = fold(I['Wd'][l], I['Ad'][l], I['Bd'][l])
        for r in range(NCORES):
            qs = Wq[r * DSH:(r + 1) * DSH]
            ks = Wk[r * HD:(r + 1) * HD]
            vs = Wv[r * HD:(r + 1) * HD]
            wqkv = np.vstack([qs, ks, vs]).T                 # (D, 768)
            per_core[r][f"wqkv{l}"] = _f16(np.ascontiguousarray(
                wqkv.reshape(KT, 128, 6, 128).transpose(2, 1, 0, 3)
                .reshape(6, 128, KT * 128)))
            wo = Wo[:, r * DSH:(r + 1) * DSH].T              # (512, D)
            per_core[r][f"wo{l}"] = _f16(np.ascontiguousarray(
                wo.reshape(QH, 128, KT, 128).transpose(2, 1, 0, 3)
                .reshape(KT, 128, QH * 128)))
            gsh = Wg[r * FSH:(r + 1) * FSH]
            ush = Wu[r * FSH:(r + 1) * FSH]
            gT = gsh.T.reshape(KT, 128, FT, 128).transpose(2, 1, 0, 3).reshape(FT, 128, KT * 128)
            uT = ush.T.reshape(KT, 128, FT, 128).transpose(2, 1, 0, 3).reshape(FT, 128, KT * 128)
            per_core[r][f"wgu{l}"] = _f16(np.ascontiguousarray(np.stack([gT, uT], axis=1)))
            wdsh = Wd[:, r * FSH:(r + 1) * FSH].T            # (1792, D)
            per_core[r][f"wd{l}"] = _f16(np.ascontiguousarray(
                wdsh.reshape(FT, 128, KT, 128).transpose(2, 1, 0, 3)
                .reshape(KT, 128, FT * 128)))
    return per_core


def kernel(**inputs):
    global _PROGRAM
    from concourse.bass_utils import run_bass_kernel_spmd

    in_maps = _host_prep(inputs)
    if _PROGRAM is None:
        _PROGRAM = _build_program()
    res = None
    for attempt in range(3):
        try:
            res = run_bass_kernel_spmd(_PROGRAM, in_maps, list(range(NCORES)))
            break
        except Exception as e:
            if attempt == 2 or 'UNAVAILABLE' not in str(type(e).__name__) + str(e):
                raise
    out = np.asarray(res.results[0]["out"]).astype(np.float32)
    return np.ascontiguousarray(out.reshape(D, B, S).transpose(1, 2, 0))
